# revision 14
# baseline (speedup 1.0000x reference)
"""DoRA Linear on 8 Trainium2 NeuronCores (Bass/Tile), fp16 + fp8-DoubleRow.

Reference computation (all fp32):
    new_v   = base_weight + SCALE * dora_B @ dora_A          [OUT, IN]
    scale_o = weight_m / ||new_v||_row                        [OUT]
    out     = x @ (scale_o[:, None] * new_v)^T + base_bias    [B, S, OUT]

Sharding: column-parallel over OUT across 8 cores (OUT/8 = 512 each).
base_weight, dora_B, weight_m, base_bias sharded; x, dora_A replicated.

The main matmul runs in mixed precision to stay under the 2e-2 rel-err
budget while using fp8 DoubleRow (2x PE throughput) where possible:
  - k-chunks 0..23 (3072 of 4096 contraction): fp16 weights and x.
  - k-chunks 24..31: e4m3 fp8, perf_mode=DoubleRow, two 128-k chunks
    per matmul.  Measured error of this 24/8 split: 1.88e-2 (9 fp8
    chunks would be 1.99e-2 - too close to the gate).
Everything is pre-scaled so both parts accumulate in one PSUM group:
weights carry x1024, x carries x16; the /16384 rides in scale_o.

Per-core device program:
  1. Build W'^T = (1024 W + 2048 B@A)^T chunk-by-chunk: PE matmul
     A^T@(2048 B^T) -> PSUM, DVE adds the fp16 1024*W^T chunk, writing
     fp16 wr16 (k<24) or fp8 wr8 pairs (k>=24).
  2. Row norms of the QUANTIZED scaled weights: ACT computes
     sq8 = Square(wr * 2^-5) into e4m3 pairs, PE accumulates
     ones8^T @ sq8 with DoubleRow norm matmuls (one PSUM group, 16 MMs).
     norm matmuls keep M = 128 output partitions (smaller M compiles
     but the runtime refuses the NEFF).
  3. scale_col = (wm/512) / sqrt(nr): PE transpose lands the norms on
     o-partitions, ACT sqrt, DVE reciprocal/mul.  (nr = norm2 * 2^-10.)
  4. Main matmul outT[o, m] = sum_k wr[k, o] * xs[k, m]: 24 fp16 MMs +
     4 fp8 DoubleRow MMs per PSUM group; eviction fuses *scale_o +
     bias_o in one DVE tensor_scalar, output stored fp16 (host upcasts).
Scheduling: m-chunk 0 matmuls are interleaved into the weight-prep loop
per k-pair (ba/ba/4+4 mc0 MMs/norm) so the PE never waits on the DVE
add chain.  All input DMAs ride one (sync) queue, ordered
critical-first with wv/xt0 interleaved in half-quarters; outputs also
ride the sync queue - SWDGE (gpsimd) stores cost a ~7us queue-teardown
DRAIN in the kernel tail.  The last m-chunk runs oc-outer so the tail
is one eviction + one small fp16 store.
Host: layout transposes + dtype casts in numpy, final gather/transpose.
"""

import numpy as np
import ml_dtypes

import concourse.mybir as mybir
import concourse.tile as tile
from concourse import bacc
from concourse.bass_utils import run_bass_kernel_spmd
from concourse.masks import make_identity

OUT, IN, RANK = 4096, 4096, 16
SCALE = 2.0
NCORES = 8
OSH = OUT // NCORES          # 512 out features per core
P = 128
KO = IN // P                 # 32 k-chunks
KO16 = 24                    # fp16 k-chunks
KP8 = (KO - KO16) // 2       # 4 fp8 k-pairs (DoubleRow)
KQ = 4                       # k-quarters of 8 chunks (3 fp16 + 1 fp8)
KO_Q = 8
M = 4 * 2048                 # 8192 tokens
MCH = 512                    # tokens per x tile
NM = M // MCH                # 16 m-chunks
OC = OSH // P                # 4 o-chunks of 128
SW = 1024.0                  # weight pre-scale (host)
SX = 16.0                    # x pre-scale (host)
SQS = 2.0 ** -5              # ACT scale for squares: sq = wr^2 * 2^-10

F32 = mybir.dt.float32
F16 = mybir.dt.float16
F8 = mybir.dt.float8e4
DR = mybir.MatmulPerfMode.DoubleRow
ADD = mybir.AluOpType.add
MULT = mybir.AluOpType.mult


def _build():
    nc = bacc.Bacc(None, target_bir_lowering=False)
    xT16 = nc.dram_tensor("xT16", [P, KO16, M], F16, kind="ExternalInput")
    xT8 = nc.dram_tensor("xT8", [P, KP8, 2, M], F8, kind="ExternalInput")
    wT = nc.dram_tensor("wT", [P, KO, OSH], F16, kind="ExternalInput")
    aT = nc.dram_tensor("aT", [RANK, IN], F16, kind="ExternalInput")
    bT = nc.dram_tensor("bT", [RANK, OSH], F32, kind="ExternalInput")
    wm = nc.dram_tensor("wm", [P, OC], F32, kind="ExternalInput")
    bc = nc.dram_tensor("bc", [P, OC], F32, kind="ExternalInput")
    outT = nc.dram_tensor("outT", [OSH, M], F16, kind="ExternalOutput")
    outT_v = outT.ap().rearrange("(oc p) m -> oc p m", p=P)

    with tile.TileContext(nc) as tc:
        with (
            tc.tile_pool(name="wr", bufs=1) as wrpool,
            tc.tile_pool(name="const", bufs=1) as cpool,
            tc.tile_pool(name="wv", bufs=2) as wvpool,
            tc.tile_pool(name="sq", bufs=3) as sqpool,
            tc.tile_pool(name="xs", bufs=6) as xpool,
            tc.tile_pool(name="os", bufs=4) as opool,
            tc.tile_pool(name="ps_mm", bufs=8, space="PSUM") as ps_mm,
        ):
            # ---- critical-first loads (single sync DMA queue) ----
            bt_f = cpool.tile([RANK, OSH], F32)
            nc.sync.dma_start(bt_f[:], bT.ap())
            at_s = cpool.tile([RANK, IN], F16)
            nc.sync.dma_start(at_s[:], aT.ap())
            # the whole fp8 x panel is only 64KB/partition - keep it
            # SBUF-resident (loaded once on the otherwise-idle gpsimd
            # queue) so no DoubleRow matmul ever waits on a DMA gate
            xr8 = cpool.tile([P, KP8, 2, M], F8)
            for kp in range(KP8):
                nc.gpsimd.dma_start(xr8[:, kp], xT8.ap()[:, kp])
            ones_f = cpool.tile([P, 2 * P], F32)
            nc.any.memset(ones_f[:], 1.0)
            # DVE order matters: ones8 first (no DMA dep) so warm-up
            # matmuls can issue while bt_f is still in flight
            ones8 = cpool.tile([P, 2, P], F8)
            nc.vector.tensor_copy(ones8[:], ones_f[:])
            bt2 = cpool.tile([RANK, OSH], F16)
            nc.vector.tensor_scalar_mul(bt2[:], bt_f[:], SCALE * SW)
            wm_col = cpool.tile([P, OC], F32)
            bias_col = cpool.tile([P, OC], F32)

            # ---- weight prep + m-chunk 0, interleaved per k-pair:
            # wr[:, ko] = 1024*W^T chunk + (2048 B A)^T chunk ----
            wr16 = wrpool.tile([P, KO16, OSH], F16)
            wr8 = wrpool.tile([P, KP8, 2, OSH], F8)
            nr = ps_mm.tile([P, OSH], F32, name="mm")
            # HAM warm-up: ~3.5us of dummy matmuls on ones8 so the PE
            # clock-gate releases (1.2 -> 2.4 GHz) before the real
            # instruction stream begins
            for _ in range(36):
                nc.tensor.matmul(nr[:, 0:P], ones8[:, 0], ones8[:, 0],
                                 start=True, stop=True)
            pss0 = [ps_mm.tile([P, MCH], F32, name="mm") for _ in range(OC)]
            pend_sq = None  # norm matmuls trail one pair behind so the
            # PE never stalls on the DVE-add -> ACT-square chain
            for kq in range(KQ):
                # interleave wv/xt0 half-quarter DMAs so the x tile
                # lands while the first chunks' adds are still running
                wv = wvpool.tile([P, KO_Q, OSH], F16)
                if kq < 3:
                    xt0 = xpool.tile([P, KO_Q, MCH], F16, name="xt")
                    nc.sync.dma_start(wv[:, 0:4], wT.ap()[:, kq * KO_Q:kq * KO_Q + 4])
                    nc.sync.dma_start(
                        xt0[:, 0:4],
                        xT16.ap()[:, kq * KO_Q:kq * KO_Q + 4, 0:MCH])
                    nc.sync.dma_start(
                        wv[:, 4:8], wT.ap()[:, kq * KO_Q + 4:(kq + 1) * KO_Q])
                    nc.sync.dma_start(
                        xt0[:, 4:8],
                        xT16.ap()[:, kq * KO_Q + 4:(kq + 1) * KO_Q, 0:MCH])
                else:
                    nc.sync.dma_start(wv[:, 0:4], wT.ap()[:, KO16:KO16 + 4])
                    nc.sync.dma_start(wv[:, 4:8], wT.ap()[:, KO16 + 4:KO])
                if kq == 0:
                    # small, needed only at scale_col time
                    nc.sync.dma_start(wm_col[:], wm.ap())
                    nc.sync.dma_start(bias_col[:], bc.ap())

                for jp in range(KO_Q // 2):
                    sq8 = sqpool.tile([P, 2, OSH], F8, name="sq8")
                    for t in range(2):
                        k8 = 2 * jp + t
                        ko = kq * KO_Q + k8
                        ba = ps_mm.tile([P, OSH], F32, name="mm")
                        nc.tensor.matmul(
                            ba[:], at_s[:, ko * P:(ko + 1) * P], bt2[:],
                            start=True, stop=True)
                        if ko < KO16:
                            wdst = wr16[:, ko]
                        else:
                            wdst = wr8[:, (ko - KO16) // 2, (ko - KO16) % 2]
                        nc.vector.tensor_tensor(wdst, wv[:, k8], ba[:], ADD)
                        nc.scalar.activation(
                            sq8[:, t], wdst,
                            mybir.ActivationFunctionType.Square, scale=SQS)
                    pair = kq * (KO_Q // 2) + jp
                    if pend_sq is not None:
                        nc.tensor.matmul(
                            nr[:], ones8[:], pend_sq[:],
                            start=(pair == 1), stop=False, perf_mode=DR)
                    pend_sq = sq8
                    # m-chunk 0 matmuls for this pair
                    if kq < 3:
                        for t in range(2):
                            k8 = 2 * jp + t
                            ko = kq * KO_Q + k8
                            for oc in range(OC):
                                nc.tensor.matmul(
                                    pss0[oc][:],
                                    wr16[:, ko, oc * P:(oc + 1) * P],
                                    xt0[:, k8],
                                    start=(ko == 0), stop=False)
                    else:
                        for oc in range(OC):
                            nc.tensor.matmul(
                                pss0[oc][:],
                                wr8[:, jp, :, oc * P:(oc + 1) * P],
                                xr8[:, jp, :, 0:MCH],
                                start=False, stop=(jp == KP8 - 1),
                                perf_mode=DR)
            nc.tensor.matmul(
                nr[:], ones8[:], pend_sq[:],
                start=False, stop=True, perf_mode=DR)

            # ---- scale_col = (wm/512) / sqrt(nr): every row of nr
            # holds the same 512 norms; PE-transpose 128-wide chunks to
            # land them on o-partitions (no DRAM bounce - that path
            # yields a NEFF the runtime refuses to load) ----
            ident = cpool.tile([P, P], F32)
            make_identity(nc, ident)
            sqc = cpool.tile([P, OC], F32)
            for oc in range(OC):
                nr_sb = sqpool.tile([P, P], F32, name="nrb")
                nc.vector.tensor_copy(nr_sb[:], nr[:, oc * P:(oc + 1) * P])
                pt = ps_mm.tile([P, P], F32, name="mm")
                nc.tensor.transpose(pt[:], nr_sb[:], ident[:])
                nc.scalar.activation(
                    sqc[:, oc:oc + 1], pt[:, 0:1],
                    mybir.ActivationFunctionType.Sqrt)
            rcp = cpool.tile([P, OC], F32)
            nc.vector.reciprocal(rcp[:], sqc[:])
            scale_col = cpool.tile([P, OC], F32)
            nc.vector.tensor_tensor(scale_col[:], wm_col[:], rcp[:], MULT)

            # ---- m-chunk 0 eviction ----
            for oc in range(OC):
                ot0 = opool.tile([P, MCH], F16, name="ot")
                nc.vector.tensor_scalar(
                    ot0[:], pss0[oc][:],
                    scale_col[:, oc:oc + 1], bias_col[:, oc:oc + 1],
                    MULT, ADD)
                nc.sync.dma_start(outT_v[oc, :, 0:MCH], ot0[:])

            # ---- main matmul: outT[o, m] accumulated over k ----
            for mc in range(1, NM):
                pss = [ps_mm.tile([P, MCH], F32, name="mm")
                       for _ in range(OC)]
                xts = []
                for kq in range(3):
                    xt = xpool.tile([P, KO_Q, MCH], F16, name="xt")
                    nc.sync.dma_start(
                        xt[:],
                        xT16.ap()[:, kq * KO_Q:(kq + 1) * KO_Q,
                                  mc * MCH:(mc + 1) * MCH])
                    xts.append(xt)

                if mc < NM - 1:
                    for kq in range(3):
                        for oc in range(OC):
                            for k8 in range(KO_Q):
                                nc.tensor.matmul(
                                    pss[oc][:],
                                    wr16[:, kq * KO_Q + k8,
                                         oc * P:(oc + 1) * P],
                                    xts[kq][:, k8],
                                    start=(kq == 0 and k8 == 0), stop=False)
                    for oc in range(OC):
                        for kp in range(KP8):
                            nc.tensor.matmul(
                                pss[oc][:],
                                wr8[:, kp, :, oc * P:(oc + 1) * P],
                                xr8[:, kp, :, mc * MCH:(mc + 1) * MCH],
                                start=False, stop=(kp == KP8 - 1),
                                perf_mode=DR)
                    for oc in range(OC):
                        ot = opool.tile([P, MCH], F16)
                        nc.vector.tensor_scalar(
                            ot[:], pss[oc][:],
                            scale_col[:, oc:oc + 1], bias_col[:, oc:oc + 1],
                            MULT, ADD)
                        nc.sync.dma_start(
                            outT_v[oc, :, mc * MCH:(mc + 1) * MCH], ot[:])
                else:
                    # last m-chunk: oc-outer so each oc's accumulation
                    # closes early and eviction/store overlap the
                    # remaining groups - shortens the kernel tail
                    for oc in range(OC):
                        for kq in range(3):
                            for k8 in range(KO_Q):
                                nc.tensor.matmul(
                                    pss[oc][:],
                                    wr16[:, kq * KO_Q + k8,
                                         oc * P:(oc + 1) * P],
                                    xts[kq][:, k8],
                                    start=(kq == 0 and k8 == 0), stop=False)
                        for kp in range(KP8):
                            nc.tensor.matmul(
                                pss[oc][:],
                                wr8[:, kp, :, oc * P:(oc + 1) * P],
                                xr8[:, kp, :, mc * MCH:(mc + 1) * MCH],
                                start=False, stop=(kp == KP8 - 1),
                                perf_mode=DR)
                        ot = opool.tile([P, MCH], F16)
                        nc.vector.tensor_scalar(
                            ot[:], pss[oc][:],
                            scale_col[:, oc:oc + 1], bias_col[:, oc:oc + 1],
                            MULT, ADD)
                        nc.sync.dma_start(
                            outT_v[oc, :, mc * MCH:(mc + 1) * MCH], ot[:])
    nc.compile()
    return nc


def kernel(x, base_weight, base_bias, weight_m, dora_A, dora_B):
    x = np.asarray(x, dtype=np.float32)
    base_weight = np.asarray(base_weight, dtype=np.float32)
    base_bias = np.asarray(base_bias, dtype=np.float32)
    weight_m = np.asarray(weight_m, dtype=np.float32)
    dora_A = np.asarray(dora_A, dtype=np.float32)
    dora_B = np.asarray(dora_B, dtype=np.float32)

    B, S, _ = x.shape
    assert B * S == M and x.shape[2] == IN

    # x layouts (shared across all cores), pre-scaled by 16:
    #   xT16[p, ko, m] = 16*x[m, ko*128+p]          fp16, ko < 24
    #   xT8[p, kp, t, m] = q8(16*x[m, (24+2kp+t)*128+p])  e4m3
    xs = (x.reshape(M, KO, P) * SX)
    xT16 = np.ascontiguousarray(
        xs[:, :KO16].transpose(2, 1, 0)).astype(np.float16)
    x8part = xs[:, KO16:].reshape(M, KP8, 2, P).transpose(3, 1, 2, 0)
    xT8 = np.clip(np.ascontiguousarray(x8part), -240, 240).astype(
        ml_dtypes.float8_e4m3)

    in_maps = []
    for c in range(NCORES):
        sl = slice(c * OSH, (c + 1) * OSH)
        w_c = base_weight[sl] * SW                              # [OSH, IN]
        wT_c = np.ascontiguousarray(
            w_c.reshape(OSH, KO, P).transpose(2, 1, 0)).astype(np.float16)
        bT_c = np.ascontiguousarray(dora_B[sl].T)               # [RANK, OSH]
        wm_c = np.ascontiguousarray(
            (weight_m[sl] / (SX * 32.0)).reshape(OC, P).T)
        bc_c = np.ascontiguousarray(base_bias[sl].reshape(OC, P).T)
        in_maps.append({
            "xT16": xT16,
            "xT8": xT8,
            "wT": wT_c,
            "aT": dora_A.astype(np.float16),
            "bT": bT_c,
            "wm": wm_c,
            "bc": bc_c,
        })

    nc = _build()
    res = run_bass_kernel_spmd(nc, in_maps, core_ids=list(range(NCORES)))

    full = np.empty((OUT, M), dtype=np.float32)
    for c in range(NCORES):
        full[c * OSH:(c + 1) * OSH] = res.results[c]["outT"].astype(
            np.float32)
    return np.ascontiguousarray(full.T).reshape(B, S, OUT)


# revision 17
# speedup vs baseline: 1.0153x; 1.0153x over previous
"""DoRA Linear on 8 Trainium2 NeuronCores (Bass/Tile), fp16 + fp8-DoubleRow.

Reference computation (all fp32):
    new_v   = base_weight + SCALE * dora_B @ dora_A          [OUT, IN]
    scale_o = weight_m / ||new_v||_row                        [OUT]
    out     = x @ (scale_o[:, None] * new_v)^T + base_bias    [B, S, OUT]

Sharding: column-parallel over OUT across 8 cores (OUT/8 = 512 each).
base_weight, dora_B, weight_m, base_bias sharded; x, dora_A replicated.

The main matmul runs in mixed precision to stay under the 2e-2 rel-err
budget while using fp8 DoubleRow (2x PE throughput) where possible:
  - k-chunks 0..23 (3072 of 4096 contraction): fp16 weights and x.
  - k-chunks 24..31: e4m3 fp8, perf_mode=DoubleRow, two 128-k chunks
    per matmul.  Measured error of this 24/8 split: 1.88e-2 (9 fp8
    chunks would be 1.99e-2 - too close to the gate).
Everything is pre-scaled so both parts accumulate in one PSUM group:
weights carry x1024, x carries x16; the /16384 rides in scale_o.

Per-core device program:
  1. Build W'^T = (1024 W + 2048 B@A)^T chunk-by-chunk: PE matmul
     A^T@(2048 B^T) -> PSUM, DVE adds the fp16 1024*W^T chunk, writing
     fp16 wr16 (k<24) or fp8 wr8 pairs (k>=24).
  2. Row norms of the QUANTIZED scaled weights: ACT computes
     sq8 = Square(wr * 2^-5) into e4m3 pairs, PE accumulates
     ones8^T @ sq8 with DoubleRow norm matmuls (one PSUM group, 16 MMs).
     norm matmuls keep M = 128 output partitions (smaller M compiles
     but the runtime refuses the NEFF).
  3. scale_col = (wm/512) / sqrt(nr): PE transpose lands the norms on
     o-partitions, ACT sqrt, DVE reciprocal/mul.  (nr = norm2 * 2^-10.)
  4. Main matmul outT[o, m] = sum_k wr[k, o] * xs[k, m]: 24 fp16 MMs +
     4 fp8 DoubleRow MMs per PSUM group; eviction fuses *scale_o +
     bias_o in one DVE tensor_scalar, output stored fp16 (host upcasts).
Scheduling: m-chunk 0 matmuls are interleaved into the weight-prep loop
per k-pair (ba/ba/4+4 mc0 MMs/norm) so the PE never waits on the DVE
add chain.  All input DMAs ride one (sync) queue, ordered
critical-first with wv/xt0 interleaved in half-quarters; outputs also
ride the sync queue - SWDGE (gpsimd) stores cost a ~7us queue-teardown
DRAIN in the kernel tail.  The last m-chunk runs oc-outer so the tail
is one eviction + one small fp16 store.
Host: layout transposes + dtype casts in numpy, final gather/transpose.
"""

import numpy as np
import ml_dtypes

import concourse.mybir as mybir
import concourse.tile as tile
from concourse import bacc
from concourse.bass_utils import run_bass_kernel_spmd
from concourse.masks import make_identity

OUT, IN, RANK = 4096, 4096, 16
SCALE = 2.0
NCORES = 8
OSH = OUT // NCORES          # 512 out features per core
P = 128
KO = IN // P                 # 32 k-chunks
KO16 = 24                    # fp16 k-chunks
KP8 = (KO - KO16) // 2       # 4 fp8 k-pairs (DoubleRow)
KQ = 4                       # k-quarters of 8 chunks (3 fp16 + 1 fp8)
KO_Q = 8
M = 4 * 2048                 # 8192 tokens
MCH = 512                    # tokens per x tile
NM = M // MCH                # 16 m-chunks
OC = OSH // P                # 4 o-chunks of 128
SW = 1024.0                  # weight pre-scale (host)
SX = 16.0                    # x pre-scale (host)
SQS = 2.0 ** -5              # ACT scale for squares: sq = wr^2 * 2^-10

F32 = mybir.dt.float32
F16 = mybir.dt.float16
F8 = mybir.dt.float8e4
DR = mybir.MatmulPerfMode.DoubleRow
ADD = mybir.AluOpType.add
MULT = mybir.AluOpType.mult


def _build():
    nc = bacc.Bacc(None, target_bir_lowering=False)
    xT16 = nc.dram_tensor("xT16", [P, KO16, M], F16, kind="ExternalInput")
    xT8 = nc.dram_tensor("xT8", [P, KP8, 2, M], F8, kind="ExternalInput")
    wT = nc.dram_tensor("wT", [P, KO, OSH], F16, kind="ExternalInput")
    aT = nc.dram_tensor("aT", [RANK, IN], F16, kind="ExternalInput")
    bT = nc.dram_tensor("bT", [RANK, OSH], F32, kind="ExternalInput")
    wm = nc.dram_tensor("wm", [P, OC], F32, kind="ExternalInput")
    bc = nc.dram_tensor("bc", [P, OC], F32, kind="ExternalInput")
    outT = nc.dram_tensor("outT", [OSH, M], F16, kind="ExternalOutput")
    outT_v = outT.ap().rearrange("(oc p) m -> oc p m", p=P)

    with tile.TileContext(nc) as tc:
        with (
            tc.tile_pool(name="wr", bufs=1) as wrpool,
            tc.tile_pool(name="const", bufs=1) as cpool,
            tc.tile_pool(name="wv", bufs=2) as wvpool,
            tc.tile_pool(name="sq", bufs=3) as sqpool,
            tc.tile_pool(name="xs", bufs=6) as xpool,
            tc.tile_pool(name="os", bufs=4) as opool,
            tc.tile_pool(name="ps_mm", bufs=8, space="PSUM") as ps_mm,
        ):
            # ---- critical-first loads (single sync DMA queue) ----
            bt_f = cpool.tile([RANK, OSH], F32)
            nc.sync.dma_start(bt_f[:], bT.ap())
            at_s = cpool.tile([RANK, IN], F16)
            nc.sync.dma_start(at_s[:], aT.ap())
            # the whole fp8 x panel is only 64KB/partition - keep it
            # SBUF-resident so no DoubleRow matmul ever waits on a DMA
            # gate.  Loads ride the sync queue AFTER the prep-critical
            # pushes (a parallel-queue load hogs the shared HW DMA
            # engines and starves the prep stream for ~24us).
            xr8 = cpool.tile([P, KP8, 2, M], F8)
            ones_f = cpool.tile([P, 2 * P], F32)
            nc.any.memset(ones_f[:], 1.0)
            # DVE order matters: ones8 first (no DMA dep) so warm-up
            # matmuls can issue while bt_f is still in flight
            ones8 = cpool.tile([P, 2, P], F8)
            nc.vector.tensor_copy(ones8[:], ones_f[:])
            bt2 = cpool.tile([RANK, OSH], F16)
            nc.vector.tensor_scalar_mul(bt2[:], bt_f[:], SCALE * SW)
            wm_col = cpool.tile([P, OC], F32)
            bias_col = cpool.tile([P, OC], F32)

            # ---- weight prep + m-chunk 0, interleaved per k-pair:
            # wr[:, ko] = 1024*W^T chunk + (2048 B A)^T chunk ----
            wr16 = wrpool.tile([P, KO16, OSH], F16)
            wr8 = wrpool.tile([P, KP8, 2, OSH], F8)
            nr = ps_mm.tile([P, OSH], F32, name="mm")
            # HAM warm-up: ~3.5us of dummy matmuls on ones8 so the PE
            # clock-gate releases (1.2 -> 2.4 GHz) before the real
            # instruction stream begins
            for _ in range(36):
                nc.tensor.matmul(nr[:, 0:P], ones8[:, 0], ones8[:, 0],
                                 start=True, stop=True)
            pss0 = [ps_mm.tile([P, MCH], F32, name="mm") for _ in range(OC)]
            pend_sq = None  # norm matmuls trail one pair behind so the
            # PE never stalls on the DVE-add -> ACT-square chain
            for kq in range(KQ):
                # interleave wv/xt0 half-quarter DMAs so the x tile
                # lands while the first chunks' adds are still running
                wv = wvpool.tile([P, KO_Q, OSH], F16)
                if kq < 3:
                    xt0 = xpool.tile([P, KO_Q, MCH], F16, name="xt")
                    nc.sync.dma_start(wv[:, 0:4], wT.ap()[:, kq * KO_Q:kq * KO_Q + 4])
                    nc.sync.dma_start(
                        xt0[:, 0:4],
                        xT16.ap()[:, kq * KO_Q:kq * KO_Q + 4, 0:MCH])
                    nc.sync.dma_start(
                        wv[:, 4:8], wT.ap()[:, kq * KO_Q + 4:(kq + 1) * KO_Q])
                    nc.sync.dma_start(
                        xt0[:, 4:8],
                        xT16.ap()[:, kq * KO_Q + 4:(kq + 1) * KO_Q, 0:MCH])
                else:
                    nc.sync.dma_start(wv[:, 0:4], wT.ap()[:, KO16:KO16 + 4])
                    nc.sync.dma_start(wv[:, 4:8], wT.ap()[:, KO16 + 4:KO])
                if kq == 0:
                    # small, needed only at scale_col time
                    nc.sync.dma_start(wm_col[:], wm.ap())
                    nc.sync.dma_start(bias_col[:], bc.ap())
                    # m-chunk 0 slice of the fp8 x panel (needed ~45us)
                    for kp in range(KP8):
                        nc.sync.dma_start(xr8[:, kp, :, 0:MCH],
                                          xT8.ap()[:, kp, :, 0:MCH])

                for jp in range(KO_Q // 2):
                    sq8 = sqpool.tile([P, 2, OSH], F8, name="sq8")
                    for t in range(2):
                        k8 = 2 * jp + t
                        ko = kq * KO_Q + k8
                        ba = ps_mm.tile([P, OSH], F32, name="mm")
                        nc.tensor.matmul(
                            ba[:], at_s[:, ko * P:(ko + 1) * P], bt2[:],
                            start=True, stop=True)
                        if ko < KO16:
                            wdst = wr16[:, ko]
                        else:
                            wdst = wr8[:, (ko - KO16) // 2, (ko - KO16) % 2]
                        nc.vector.tensor_tensor(wdst, wv[:, k8], ba[:], ADD)
                        nc.scalar.activation(
                            sq8[:, t], wdst,
                            mybir.ActivationFunctionType.Square, scale=SQS)
                    pair = kq * (KO_Q // 2) + jp
                    if pend_sq is not None:
                        nc.tensor.matmul(
                            nr[:], ones8[:], pend_sq[:],
                            start=(pair == 1), stop=False, perf_mode=DR)
                    pend_sq = sq8
                    # m-chunk 0 matmuls for this pair
                    if kq < 3:
                        for t in range(2):
                            k8 = 2 * jp + t
                            ko = kq * KO_Q + k8
                            for oc in range(OC):
                                nc.tensor.matmul(
                                    pss0[oc][:],
                                    wr16[:, ko, oc * P:(oc + 1) * P],
                                    xt0[:, k8],
                                    start=(ko == 0), stop=False)
                    else:
                        for oc in range(OC):
                            nc.tensor.matmul(
                                pss0[oc][:],
                                wr8[:, jp, :, oc * P:(oc + 1) * P],
                                xr8[:, jp, :, 0:MCH],
                                start=False, stop=(jp == KP8 - 1),
                                perf_mode=DR)
            nc.tensor.matmul(
                nr[:], ones8[:], pend_sq[:],
                start=False, stop=True, perf_mode=DR)
            # bulk of the fp8 x panel: after the prep stream, before
            # the main-loop x tiles (all on the ordered sync queue)
            for kp in range(KP8):
                nc.sync.dma_start(xr8[:, kp, :, MCH:M],
                                  xT8.ap()[:, kp, :, MCH:M])

            # ---- scale_col = (wm/512) / sqrt(nr): every row of nr
            # holds the same 512 norms; PE-transpose 128-wide chunks to
            # land them on o-partitions (no DRAM bounce - that path
            # yields a NEFF the runtime refuses to load) ----
            ident = cpool.tile([P, P], F32)
            make_identity(nc, ident)
            sqc = cpool.tile([P, OC], F32)
            for oc in range(OC):
                nr_sb = sqpool.tile([P, P], F32, name="nrb")
                nc.vector.tensor_copy(nr_sb[:], nr[:, oc * P:(oc + 1) * P])
                pt = ps_mm.tile([P, P], F32, name="mm")
                nc.tensor.transpose(pt[:], nr_sb[:], ident[:])
                nc.scalar.activation(
                    sqc[:, oc:oc + 1], pt[:, 0:1],
                    mybir.ActivationFunctionType.Sqrt)
            rcp = cpool.tile([P, OC], F32)
            nc.vector.reciprocal(rcp[:], sqc[:])
            scale_col = cpool.tile([P, OC], F32)
            nc.vector.tensor_tensor(scale_col[:], wm_col[:], rcp[:], MULT)

            # ---- m-chunk 0 eviction ----
            for oc in range(OC):
                ot0 = opool.tile([P, MCH], F16, name="ot")
                nc.vector.tensor_scalar(
                    ot0[:], pss0[oc][:],
                    scale_col[:, oc:oc + 1], bias_col[:, oc:oc + 1],
                    MULT, ADD)
                nc.sync.dma_start(outT_v[oc, :, 0:MCH], ot0[:])

            # ---- main matmul: outT[o, m] accumulated over k ----
            for mc in range(1, NM):
                pss = [ps_mm.tile([P, MCH], F32, name="mm")
                       for _ in range(OC)]
                xts = []
                for kq in range(3):
                    xt = xpool.tile([P, KO_Q, MCH], F16, name="xt")
                    nc.sync.dma_start(
                        xt[:],
                        xT16.ap()[:, kq * KO_Q:(kq + 1) * KO_Q,
                                  mc * MCH:(mc + 1) * MCH])
                    xts.append(xt)

                if mc < NM - 1:
                    for kq in range(3):
                        for oc in range(OC):
                            for k8 in range(KO_Q):
                                nc.tensor.matmul(
                                    pss[oc][:],
                                    wr16[:, kq * KO_Q + k8,
                                         oc * P:(oc + 1) * P],
                                    xts[kq][:, k8],
                                    start=(kq == 0 and k8 == 0), stop=False)
                    for oc in range(OC):
                        for kp in range(KP8):
                            nc.tensor.matmul(
                                pss[oc][:],
                                wr8[:, kp, :, oc * P:(oc + 1) * P],
                                xr8[:, kp, :, mc * MCH:(mc + 1) * MCH],
                                start=False, stop=(kp == KP8 - 1),
                                perf_mode=DR)
                    for oc in range(OC):
                        ot = opool.tile([P, MCH], F16)
                        nc.vector.tensor_scalar(
                            ot[:], pss[oc][:],
                            scale_col[:, oc:oc + 1], bias_col[:, oc:oc + 1],
                            MULT, ADD)
                        nc.sync.dma_start(
                            outT_v[oc, :, mc * MCH:(mc + 1) * MCH], ot[:])
                else:
                    # last m-chunk: oc-outer so each oc's accumulation
                    # closes early and eviction/store overlap the
                    # remaining groups - shortens the kernel tail
                    for oc in range(OC):
                        for kq in range(3):
                            for k8 in range(KO_Q):
                                nc.tensor.matmul(
                                    pss[oc][:],
                                    wr16[:, kq * KO_Q + k8,
                                         oc * P:(oc + 1) * P],
                                    xts[kq][:, k8],
                                    start=(kq == 0 and k8 == 0), stop=False)
                        for kp in range(KP8):
                            nc.tensor.matmul(
                                pss[oc][:],
                                wr8[:, kp, :, oc * P:(oc + 1) * P],
                                xr8[:, kp, :, mc * MCH:(mc + 1) * MCH],
                                start=False, stop=(kp == KP8 - 1),
                                perf_mode=DR)
                        ot = opool.tile([P, MCH], F16)
                        nc.vector.tensor_scalar(
                            ot[:], pss[oc][:],
                            scale_col[:, oc:oc + 1], bias_col[:, oc:oc + 1],
                            MULT, ADD)
                        nc.sync.dma_start(
                            outT_v[oc, :, mc * MCH:(mc + 1) * MCH], ot[:])
    nc.compile()
    return nc


def kernel(x, base_weight, base_bias, weight_m, dora_A, dora_B):
    x = np.asarray(x, dtype=np.float32)
    base_weight = np.asarray(base_weight, dtype=np.float32)
    base_bias = np.asarray(base_bias, dtype=np.float32)
    weight_m = np.asarray(weight_m, dtype=np.float32)
    dora_A = np.asarray(dora_A, dtype=np.float32)
    dora_B = np.asarray(dora_B, dtype=np.float32)

    B, S, _ = x.shape
    assert B * S == M and x.shape[2] == IN

    # x layouts (shared across all cores), pre-scaled by 16:
    #   xT16[p, ko, m] = 16*x[m, ko*128+p]          fp16, ko < 24
    #   xT8[p, kp, t, m] = q8(16*x[m, (24+2kp+t)*128+p])  e4m3
    xs = (x.reshape(M, KO, P) * SX)
    xT16 = np.ascontiguousarray(
        xs[:, :KO16].transpose(2, 1, 0)).astype(np.float16)
    x8part = xs[:, KO16:].reshape(M, KP8, 2, P).transpose(3, 1, 2, 0)
    xT8 = np.clip(np.ascontiguousarray(x8part), -240, 240).astype(
        ml_dtypes.float8_e4m3)

    in_maps = []
    for c in range(NCORES):
        sl = slice(c * OSH, (c + 1) * OSH)
        w_c = base_weight[sl] * SW                              # [OSH, IN]
        wT_c = np.ascontiguousarray(
            w_c.reshape(OSH, KO, P).transpose(2, 1, 0)).astype(np.float16)
        bT_c = np.ascontiguousarray(dora_B[sl].T)               # [RANK, OSH]
        wm_c = np.ascontiguousarray(
            (weight_m[sl] / (SX * 32.0)).reshape(OC, P).T)
        bc_c = np.ascontiguousarray(base_bias[sl].reshape(OC, P).T)
        in_maps.append({
            "xT16": xT16,
            "xT8": xT8,
            "wT": wT_c,
            "aT": dora_A.astype(np.float16),
            "bT": bT_c,
            "wm": wm_c,
            "bc": bc_c,
        })

    nc = _build()
    res = run_bass_kernel_spmd(nc, in_maps, core_ids=list(range(NCORES)))

    full = np.empty((OUT, M), dtype=np.float32)
    for c in range(NCORES):
        full[c * OSH:(c + 1) * OSH] = res.results[c]["outT"].astype(
            np.float32)
    return np.ascontiguousarray(full.T).reshape(B, S, OUT)
